# revision 51
# baseline (speedup 1.0000x reference)
"""Trainium2 Bass kernel for nn_EncoderLayer (B=4, S=2048, D=1024, H=16, DFF=4096).

Sharding (8 cores, collective-free): core c handles batch b=c//2 and token
half g=c%2. Each core computes K and V for the full sequence (duplicated
across the pair) but Q/attention/out-proj/LayerNorms/FFN only for its own
1024 tokens, with full weights, so no cross-core reduction is needed.

All layout work happens on the HOST: x and every weight arrive
pre-transposed ([d, t] activations-on-partitions convention), attention
weights in fp8e4m3 scaled x64 (dodges the e4m3 subnormal band; evictions
fold the 1/64 back), FFN weights bf16. Q/K/V and out-proj run as fp8
DoubleRow matmuls (256-deep contraction, 0.5 cyc/row); attn@V is DoubleRow
over key-tile pairs with a ones-column in V so the softmax denominator
falls out of the same matmul; scores are plain fp8 matmuls (DK=64-deep)
whose 1/8 scale folds into the softmax exp. The attention fp8 noise washes
out through the 2048-key softmax averaging. FFN stays bf16. LayerNorm
affines fold into FFN weights / host-precomputed bias vectors; LN rstd uses
exp(-0.5*ln(var)) so every ACT op lives in one activation table (no
reloads); partition broadcasts go through small PE matmuls, never DRAM.

Issue order pipelines phases to keep PE fed under the ACT-bound softmax
window: chunk-0 attention streams first, then out-proj/LN1/FFN1 of chunk 0
interleave into chunk 1's attention blocks.
"""

import numpy as np
import ml_dtypes

import concourse.bass as bass
import concourse.mybir as mybir
import concourse.tile as tile
from concourse.bass_utils import run_bass_kernel_spmd
from concourse.vector_clock import ScopedClock

f32 = mybir.dt.float32
bf16 = mybir.dt.bfloat16
f8 = mybir.dt.float8e4
AF = mybir.ActivationFunctionType
ALU = mybir.AluOpType
DR = mybir.MatmulPerfMode.DoubleRow

P = 128
S = 2048  # tokens per batch (full sequence)
SH = 1024  # tokens owned by this core
D = 1024  # model dim
DK = 64  # head dim
H = 16  # heads
DFF = 4096
NC = 512  # matmul moving free dim
KO = D // P  # 8 contraction chunks over D
KP = KO // 2  # 4 DoubleRow pairs over D
KT = S // P  # 16 key tiles
K2 = KT // 2  # 8 key-tile pairs
NO_H = SH // NC  # 2 chunks over own tokens
NO_S = S // NC  # 4 chunks over the full sequence
JB = DFF // NC  # 8 dff blocks
FO = DFF // P  # 32
HP = H // 2  # 8 head pairs
TQ = 256  # attention/FFN token chunk (4 chunks over SH)
NO4 = SH // TQ  # 4
K4 = KT // 4  # 4 score tiles per head-parity (4 key-tiles each)
RWS = 1.0 / 64.0  # fp8 weight scale compensation
RWS2 = RWS * RWS


# ---------------------------------------------------------------------------
# Walrus in this container accepts at most ONE sync-wait command per
# instruction; Tile freely attaches several. TC overrides the exit sequence
# and legalize_single_wait splits multi-wait instructions into standalone
# EventSemaphore waits.
# ---------------------------------------------------------------------------
def legalize_single_wait(nc):
    n_split = 0
    for fn in nc.m.functions:
        for bb in fn.blocks:
            insts = bb.instructions
            i = 0
            while i < len(insts):
                ins = insts[i]
                si = ins.sync_info
                if si is not None and si.on_wait and len(si.on_wait) > 1:
                    extra = list(si.on_wait[:-1])
                    del si.on_wait[:-1]
                    for w in extra:
                        assert w.wait_mode == "sem-ge-imm", w
                        h = bass.SemaphoreHandle(w.ant_name, w.id)
                        wi = nc.engines[ins.engine].wait_ge(h, w.wait_value).ins
                        cur = nc.main_func.blocks[-1].instructions
                        assert cur[-1] is wi
                        cur.pop()
                        insts.insert(i, wi)
                        i += 1
                        n_split += 1
                i += 1
    return n_split


class TC(tile.TileContext):
    def _drain_and_barrier(self, tick_clock, wait_clock):
        nc = self.nc
        carrier = nc.sync.nop()
        wait_clock.add_sem_waits(
            carrier.ins, ScopedClock({None: tick_clock.global_clock})
        )
        waits = []
        if carrier.ins.sync_info is not None and carrier.ins.sync_info.on_wait:
            waits = list(carrier.ins.sync_info.on_wait)
            del carrier.ins.sync_info.on_wait[:]
        assert self.sems is not None
        id2h = {h.num: h for h in self.sems.allocated().values()}
        for w in waits:
            assert w.wait_mode == "sem-ge-imm", w
            h = id2h.get(w.id)
            if h is None:
                raise RuntimeError(f"unknown sem id {w.id} ({w.ant_name})")
            nc.sync.wait_ge(h, w.wait_value)
        nc.sync.drain()
        nc.all_engine_barrier(sem_only=True)
        popped = nc._tile_sem_poison_stack.pop()
        assert popped is self._sem_poison
        nc.clear_and_free_semaphores(list(self.sems.allocated().values()))
        nc.all_engine_barrier(sem_only=True)

    def __exit__(self, *exc):
        ret = super().__exit__(*exc)
        if exc[0] is None:
            legalize_single_wait(self.nc)
        return ret


def _pool(tc, **kw):
    cm = tc.tile_pool(**kw)
    return cm, cm.__enter__()


def build_nc():
    nc = bass.Bass()
    d = lambda n, shp, dt: nc.declare_dram_parameter(n, shp, dt, isOutput=False)
    xT8_ext = d("xT8", [D, S], f8)
    xh8_ext = d("xh8", [D, SH], f8)
    xhT_ext = d("xhT", [D, SH], bf16)
    wkT8_ext = d("wkT8", [D, D], f8)
    wvT8_ext = d("wvT8", [D, D], f8)
    wqT8_ext = d("wqT8", [D, D], f8)
    woT8_ext = d("woT8", [D, D], f8)
    w1T_ext = d("w1T", [D, DFF], bf16)
    w2T_ext = d("w2T", [DFF, D], bf16)
    bk_ext = d("bk", [D], f32)
    bq_ext = d("bq", [D], f32)
    bo2_ext = d("bo2", [D], f32)
    b1f_ext = d("b1f", [DFF], f32)
    be1b2_ext = d("be1b2", [D], f32)
    g1_ext = d("g1", [D], f32)
    g2_ext = d("g2", [D], f32)
    be2_ext = d("be2", [D], f32)
    out_ext = nc.declare_dram_parameter("out", [D, SH], f32, isOutput=True)
    outT = out_ext.rearrange("(o p) t -> p o t", p=P)

    with TC(nc) as tc:
        # SBUF stack: misc | actp | ffn | attp | kq | [xw -> work -> w2p]
        misc_cm, misc = _pool(tc, name="misc", bufs=1)
        actp_cm, actp = _pool(tc, name="actp", bufs=1)
        ffn_cm, ffn = _pool(tc, name="ffn", bufs=2)
        attp_cm, attp = _pool(tc, name="attp", bufs=1)
        kq_cm, kq = _pool(tc, name="kq", bufs=1)
        # PSUM: 4 + 2 + 2 banks, all open for the whole kernel
        psA_cm, psA = _pool(tc, name="psA", bufs=2, space="PSUM")
        psB_cm, psB = _pool(tc, name="psB", bufs=2, space="PSUM")
        psC_cm, psC = _pool(tc, name="psC", bufs=2, space="PSUM")

        ones_b = misc.tile([P, 1], bf16)
        nc.vector.memset(ones_b[:], 1.0)
        ones_r = misc.tile([1, P], f32)
        nc.vector.memset(ones_r[:], 1.0)
        ones64 = misc.tile([1, DK], bf16)
        nc.vector.memset(ones64[:], 64.0)  # folds the x64 ctx fp8 scale

        def load_bias(ext_ap, n, name):
            t = misc.tile([P, n // P], f32, tag=f"bias_{name}", name=f"b_{name}")
            nc.sync.dma_start(t[:], ext_ap.rearrange("(o p) -> p o", p=P))
            return t

        xhT = actp.tile([P, KO, SH], bf16, tag="resid", name="xhT")
        zT = actp.tile([P, KO, SH], bf16, tag="zT", name="zT")
        v_aug = attp.tile([P, KT, H, DK + 1], f8)
        ctxT8 = attp.tile([P, KO, SH], f8)
        kT8 = kq.tile([P, KO, S], f8)
        qT8 = kq.tile([P, KO, SH], f8)

        nc.vector.memset(v_aug[:, :, :, DK : DK + 1], 1.0)

        # ---------------- LayerNorm (shared by LN1/LN2) ----------------------
        def ln_chunk(yT, no, emit, lp=None, sfx="", post=False):
            """Generator: unbiased LN stats of yT[:, :, 256-chunk no] ->
            per-ko emit(no, ko, t1, mb); t1 = y - mean_b, mb[:, TQ:] = rstd_b.
            lp/sfx give concurrent instances disjoint pools/tag rings."""
            lp = lp or ffn
            tq = slice(no * TQ, (no + 1) * TQ)
            ps_sum = psC.tile([1, TQ], f32, tag="pc", name="ps_sum")
            for ko in range(KO):
                nc.tensor.matmul(
                    ps_sum[:],
                    ones_b[:, 0:1],
                    yT[:, ko, tq],
                    start=(ko == 0),
                    stop=(ko == KO - 1),
                )
            scr_s = lp.tile([1, TQ], bf16, tag="scr" + sfx, bufs=2, name="scr_s")
            nc.vector.tensor_copy(scr_s[:], ps_sum[:])
            yield
            ps_sq = psC.tile([1, TQ], f32, tag="pc", name="ps_sq")
            for ko in range(KO):
                sqt = lp.tile([P, TQ], bf16, tag="sq" + sfx, bufs=3, name="sqt")
                nc.vector.tensor_mul(sqt[:], yT[:, ko, tq], yT[:, ko, tq])
                nc.tensor.matmul(
                    ps_sq[:],
                    ones_b[:, 0:1],
                    sqt[:],
                    start=(ko == 0),
                    stop=(ko == KO - 1),
                )
                if ko == 3:
                    yield
            pk = lp.tile([1, 2 * TQ], f32, tag="pk" + sfx, name="pk")
            nc.vector.tensor_scalar_mul(pk[0:1, 0:TQ], scr_s[:], 1.0 / D)
            scr_m = lp.tile([1, TQ], bf16, tag="scr" + sfx, bufs=2, name="scr_m")
            nc.vector.tensor_mul(scr_m[:], pk[0:1, 0:TQ], scr_s[:])
            scr_v = lp.tile([1, TQ], bf16, tag="scr" + sfx, bufs=2, name="scr_v")
            nc.vector.tensor_sub(scr_v[:], ps_sq[:], scr_m[:])
            # rstd = (var)^-0.5 via exp(-ln/2): stays in the exp act table.
            # (+eps on std is a ~1e-6 relative tweak; folded away.)
            scr_l = lp.tile([1, TQ], bf16, tag="scr" + sfx, bufs=2, name="scr_l")
            nc.scalar.activation(scr_l[:], scr_v[:], AF.Ln, scale=1.0 / (D - 1))
            nc.scalar.activation(pk[0:1, TQ : 2 * TQ], scr_l[:], AF.Exp, scale=-0.5)
            yield
            ps_bc = psA.tile([P, 2 * TQ], f32, tag="pa", name="ps_bc")
            nc.tensor.matmul(
                ps_bc[:, 0:TQ], ones_r[:], pk[0:1, 0:TQ], start=True, stop=True
            )
            nc.tensor.matmul(
                ps_bc[:, TQ : 2 * TQ], ones_r[:], pk[0:1, TQ : 2 * TQ],
                start=True, stop=True,
            )
            mb = lp.tile([P, 2 * TQ], bf16, tag="mb" + sfx, bufs=2, name="mb")
            nc.vector.tensor_copy(mb[:], ps_bc[:])
            yield
            for ko in range(KO):
                t1 = lp.tile([P, TQ], bf16, tag="t1" + sfx, bufs=2, name="t1")
                eng = nc.gpsimd if sfx else nc.vector
                eng.tensor_sub(t1[:], yT[:, ko, tq], mb[:, 0:TQ])
                emit(no, ko, t1, mb, lp, sfx)
                if ko % 3 == 2:
                    yield

        def emit_z(no, ko, t1, mb, lp, sfx):
            tq = slice(no * TQ, (no + 1) * TQ)
            nc.vector.tensor_mul(zT[:, ko, tq], t1[:], mb[:, TQ : 2 * TQ])

        def emit_z_pool(no, ko, t1, mb, lp, sfx):
            tq = slice(no * TQ, (no + 1) * TQ)
            nc.gpsimd.tensor_mul(zT[:, ko, tq], t1[:], mb[:, TQ : 2 * TQ])

        def emit_out(no, ko, t1, mb, lp, sfx):
            tq = slice(no * TQ, (no + 1) * TQ)
            z2 = lp.tile([P, TQ], bf16, tag="t1" + sfx, bufs=2, name="z2")
            nc.vector.tensor_mul(z2[:], t1[:], mb[:, TQ : 2 * TQ])
            of = lp.tile([P, TQ], f32, tag="t2" + sfx, bufs=2, name="of")
            nc.scalar.activation(
                of[:], z2[:], AF.Identity,
                bias=be2_sb[:, ko : ko + 1], scale=g2_sb[:, ko : ko + 1],
            )
            nc.sync.dma_start(outT[:, ko, tq], of[:])

        # ---------------- Phase A: fp8 DoubleRow projections -----------------
        xw_cm, xw = _pool(tc, name="xw", bufs=2)

        def wload(ext, name):
            w = xw.tile([P, KO, D], f8, tag="wring", name=name)
            nc.sync.dma_start(w[:], ext.rearrange("(o p) n -> p o n", p=P))
            return w

        def xload(c, name):
            xc = xw.tile([P, KO, NC], f8, tag="xring", bufs=3, name=name)
            if c < NO_S:
                src = xT8_ext.rearrange("(o p) t -> p o t", p=P)
                nc.sync.dma_start(xc[:], src[:, :, c * NC : (c + 1) * NC])
            else:
                src = xh8_ext.rearrange("(o p) t -> p o t", p=P)
                nc.sync.dma_start(
                    xc[:], src[:, :, (c - NO_S) * NC : (c - NO_S + 1) * NC]
                )
            return xc

        def dr_accum(ps, lhsT3, rhs3):
            """ps += sum over 4 DoubleRow pairs; lhsT3/rhs3: kp -> AP."""
            for kp in range(KP):
                nc.tensor.matmul(
                    ps[:],
                    lhsT3(kp),
                    rhs3(kp),
                    start=(kp == 0),
                    stop=(kp == KP - 1),
                    perf_mode=DR,
                )

        wk_sb = xw.tile([P, KO, D], f8, tag="wring", name="wk_sb")
        wk_src = wkT8_ext.rearrange("(o p) n -> p o n", p=P)
        nxt = xload(0, "xk0")
        for mo in range(KO):
            nc.sync.dma_start(
                wk_sb[:, :, mo * P : (mo + 1) * P],
                wk_src[:, :, mo * P : (mo + 1) * P],
            )
        wq_sb = xw.tile([P, KO, D], f8, tag="wring", name="wq_sb")
        bk_sb = load_bias(bk_ext, D, "bk")
        # K pass (full sequence); evictions alternate ACT/DVE to keep pace
        for c in range(NO_S):
            xc = nxt
            if c < NO_S - 1:
                nxt = xload(c + 1, f"xk{c + 1}")
            if c == 2:
                nc.sync.dma_start(
                    wq_sb[:], wqT8_ext.rearrange("(o p) n -> p o n", p=P)
                )
                bq_sb = load_bias(bq_ext, D, "bq")
                bo2_sb = load_bias(bo2_ext, D, "bo2")
                b1f_sb = load_bias(b1f_ext, DFF, "b1f")
                be1b2_sb = load_bias(be1b2_ext, D, "be1b2")
                g1_sb = load_bias(g1_ext, D, "g1")
                g2_sb = load_bias(g2_ext, D, "g2")
                be2_sb = load_bias(be2_ext, D, "be2")
            for mo in range(KO):
                pool = psA if mo % 2 == 0 else psC
                tag = "pa" if mo % 2 == 0 else "pc"
                ps = pool.tile([P, NC], f32, tag=tag, name=f"ps_k{mo}")
                dr_accum(
                    ps,
                    lambda kp, mo=mo: wk_sb[:, 2 * kp : 2 * kp + 2, mo * P : (mo + 1) * P],
                    lambda kp: xc[:, 2 * kp : 2 * kp + 2, :],
                )
                if mo % 2 == 0:
                    nc.scalar.activation(
                        kT8[:, mo, c * NC : (c + 1) * NC],
                        ps[:],
                        AF.Identity,
                        bias=bk_sb[:, mo : mo + 1],
                        scale=RWS,
                    )
                else:
                    nc.vector.tensor_scalar(
                        kT8[:, mo, c * NC : (c + 1) * NC],
                        ps[:],
                        RWS,
                        bk_sb[:, mo : mo + 1],
                        ALU.mult,
                        ALU.add,
                    )
        # Q pass (own half); evictions alternate DVE/ACT
        nxt = xload(NO_S, "xq0")
        wv_sb = wload(wvT8_ext, "wv_sb")
        for c in range(NO_H):
            xc = nxt
            if c == 0:
                nxt = xload(NO_S + 1, "xq1")
                nxt_v = xload(0, "xv0")
            for mo in range(KO):
                pool = psA if mo % 2 == 0 else psC
                tag = "pa" if mo % 2 == 0 else "pc"
                ps = pool.tile([P, NC], f32, tag=tag, name=f"ps_q{mo}")
                dr_accum(
                    ps,
                    lambda kp, mo=mo: wq_sb[:, 2 * kp : 2 * kp + 2, mo * P : (mo + 1) * P],
                    lambda kp: xc[:, 2 * kp : 2 * kp + 2, :],
                )
                if mo % 2 == 0:
                    nc.vector.tensor_scalar(
                        qT8[:, mo, c * NC : (c + 1) * NC],
                        ps[:],
                        RWS,
                        bq_sb[:, mo : mo + 1],
                        ALU.mult,
                        ALU.add,
                    )
                else:
                    nc.scalar.activation(
                        qT8[:, mo, c * NC : (c + 1) * NC],
                        ps[:],
                        AF.Identity,
                        bias=bq_sb[:, mo : mo + 1],
                        scale=RWS,
                    )
        # V pass (full sequence, x as stationary)
        # bv folds into bo2 on the host via the attn@V ones-column identity.
        for c in range(NO_S):
            xc = nxt_v
            if c < NO_S - 1:
                nxt_v = xload(c + 1, f"xv{c + 1}")
            for ti in range(4):
                to = c * 4 + ti
                for nch in range(2):
                    pool = psA if nch == 0 else psC
                    tag = "pa" if nch == 0 else "pc"
                    ps = pool.tile([P, NC], f32, tag=tag, name=f"ps_v{nch}")
                    dr_accum(
                        ps,
                        lambda kp, ti=ti: xc[:, 2 * kp : 2 * kp + 2, ti * P : (ti + 1) * P],
                        lambda kp, nch=nch: wv_sb[:, 2 * kp : 2 * kp + 2, nch * NC : (nch + 1) * NC],
                    )
                    if nch == 0:
                        nc.scalar.activation(
                            v_aug[:, to, 0:8, 0:DK],
                            ps.rearrange("p (h dv) -> p h dv", h=8),
                            AF.Identity,
                            scale=RWS,
                        )
                    else:
                        nc.vector.tensor_scalar_mul(
                            v_aug[:, to, 8:16, 0:DK],
                            ps.rearrange("p (h dv) -> p h dv", h=8),
                            RWS,
                        )
        xw_cm.__exit__(None, None, None)

        # ---------------- Phase B: attention (4 chunks of 256 tokens) -------
        work_cm, work = _pool(tc, name="work", bufs=1)
        wo_sb = work.tile([P, KO, D], f8, tag="wo", name="wo_sb")
        nc.sync.dma_start(wo_sb[:], woT8_ext.rearrange("(o p) n -> p o n", p=P))
        nc.sync.dma_start(xhT[:], xhT_ext.rearrange("(o p) t -> p o t", p=P))

        def ctx_finish(no, hp, ps_pair, recs):
            tq = slice(no * TQ, (no + 1) * TQ)
            for par in range(2):
                ps_rb = psC.tile([DK, TQ], f32, tag="pc", name="ps_rb")
                nc.tensor.matmul(
                    ps_rb[:], ones64[:], recs[par][:], start=True, stop=True
                )
                recb = work.tile([DK, TQ], bf16, tag="recb", bufs=2, name="recb")
                nc.vector.tensor_copy(recb[:], ps_rb[:])
                if par == 0:
                    nc.vector.tensor_mul(
                        ctxT8[0:DK, hp, tq], ps_pair[0:DK, 0, :], recb[:]
                    )
                else:
                    ctmp = work.tile([DK, TQ], f8, tag="ctmp", bufs=2, name="ctmp")
                    nc.vector.tensor_mul(ctmp[:], ps_pair[0:DK, 1, :], recb[:])
                    nc.sync.dma_start(ctxT8[DK:P, hp, tq], ctmp[:])

        def attn_block(no, hp, prev_fin):
            # prev head-pair's normalize emits a few score tiles in, so its
            # reciprocal (issued at the prev block's end) never blocks PE
            tq = slice(no * TQ, (no + 1) * TQ)
            # both parities' ctx accumulators share one PSUM bank: one
            # accumulation group, start on the first write, stop on the last
            ps_pair = psB.tile([P, 2, TQ], f32, tag="pb", name="ps_pair")
            pend = {}
            for k4 in range(K4 + 2):
                if k4 < K4:
                    for par in range(2):
                        base = DK * par
                        ps_s = psA.tile([P, 4, TQ], f32, tag="pa", name="ps_s")
                        for i in range(4):
                            kt = 4 * k4 + i
                            nc.tensor.matmul(
                                ps_s[:, i, :],
                                kT8[base : base + DK, hp, kt * P : (kt + 1) * P],
                                qT8[base : base + DK, hp, tq],
                                start=True,
                                stop=True,
                            )
                        at = work.tile(
                            [P, 4, TQ], f8, tag="at", bufs=6, name="at"
                        )
                        nc.scalar.activation(at[:], ps_s[:], AF.Exp, scale=0.125)
                        pend[(k4, par)] = at
                if k4 == 4 and prev_fin is not None:
                    prev_fin()
                kv4 = k4 - 2
                if kv4 >= 0:
                    for par in range(2):
                        at = pend.pop((kv4, par))
                        h = 2 * hp + par
                        for i in range(2):
                            kv = 2 * kv4 + i
                            nc.tensor.matmul(
                                ps_pair[0 : DK + 1, par, :],
                                v_aug[:, 2 * kv : 2 * kv + 2, h, :],
                                at[:, 2 * i : 2 * i + 2, :],
                                start=(kv == 0 and par == 0),
                                stop=(kv == K2 - 1 and par == 1),
                                perf_mode=DR,
                                skip_group_check=True,
                            )
            recs = []
            for par in range(2):
                rec = work.tile([1, TQ], bf16, tag="rec", bufs=4, name="rec")
                with nc.allow_low_precision(reason="softmax denom recip, bf16 ok"):
                    nc.vector.reciprocal(rec[:], ps_pair[DK : DK + 1, par, :])
                recs.append(rec)
            return lambda: ctx_finish(no, hp, ps_pair, recs)

        def outproj_unit(no, mo):
            tq = slice(no * TQ, (no + 1) * TQ)
            ps = psC.tile([P, TQ], f32, tag="pc", name="ps_o")
            dr_accum(
                ps,
                lambda kp, mo=mo: wo_sb[:, 2 * kp : 2 * kp + 2, mo * P : (mo + 1) * P],
                lambda kp: ctxT8[:, 2 * kp : 2 * kp + 2, tq],
            )
            ao = ffn.tile([P, TQ], f32, tag="t2", name="ao")
            nc.vector.tensor_scalar(
                ao[:], ps[:], RWS2, bo2_sb[:, mo : mo + 1], ALU.mult, ALU.add
            )
            nc.vector.tensor_add(xhT[:, mo, tq], xhT[:, mo, tq], ao[:])

        def w1_load(j):
            w1b = ffn.tile([P, KO, NC], bf16, tag="w1blk", name=f"w1b{j}")
            src = w1T_ext.rearrange("(o p) f -> p o f", p=P)
            nc.sync.dma_start(w1b[:], src[:, :, j * NC : (j + 1) * NC])
            return w1b

        def ffn1_unit(no, j, mo, w1b, hT, relu_on_act):
            tq = slice(no * TQ, (no + 1) * TQ)
            ps = psC.tile([P, TQ], f32, tag="pc", name="ps_f1")
            for ko in range(KO):
                nc.tensor.matmul(
                    ps[:],
                    w1b[:, ko, mo * P : (mo + 1) * P],
                    zT[:, ko, tq],
                    start=(ko == 0),
                    stop=(ko == KO - 1),
                )
            col = j * 4 + mo
            if relu_on_act:
                nc.scalar.activation(
                    hT[:, col, :], ps[:], AF.Relu, bias=b1f_sb[:, col : col + 1]
                )
            else:
                nc.vector.tensor_scalar(
                    hT[:, col, :], ps[:], b1f_sb[:, col : col + 1], 0.0,
                    ALU.add, ALU.max,
                )

        hTs = [
            ffn.tile([P, FO, TQ], bf16, tag="hT", bufs=4, name=f"hT{i}")
            for i in range(NO4)
        ]

        def gen_oln(no):
            for mo in range(KO):
                outproj_unit(no, mo)
                yield
            yield from ln_chunk(xhT, no, emit_z)

        def gen_ffn1(nos):
            for j in range(JB):
                w1b = w1_load(j)
                for no in nos:
                    for mo in range(4):
                        ffn1_unit(no, j, mo, w1b, hTs[no], relu_on_act=False)
                        yield

        pending = []

        def pump(n):
            for _ in range(n):
                while pending:
                    try:
                        next(pending[0])
                        break
                    except StopIteration:
                        pending.pop(0)
                else:
                    break

        fin = None
        for no in range(NO4):
            for hp in range(HP):
                fin = attn_block(no, hp, fin)
                if no >= 1:
                    pump((0, 6, 6, 5)[no])
            if no < NO4 - 1:
                pending.append(gen_oln(no))
                pending.append(gen_ffn1((no,)))
        fin()
        pump(10**9)

        for mo in range(KO):
            outproj_unit(3, mo)
        for _ in ln_chunk(xhT, 3, emit_z):
            pass

        work_cm.__exit__(None, None, None)
        kq_cm.__exit__(None, None, None)
        attp_cm.__exit__(None, None, None)

        # ---------------- Phase C: FFN1(ch3) + streamed-w2 FFN2 + LN2 --------
        w2p_cm, w2p = _pool(tc, name="w2p", bufs=1)
        w2src = w2T_ext.rearrange("(o p) n -> p o n", p=P)
        x2a = w2p.tile([P, KO, SH], f32, tag="x2a", name="x2a")
        x2b = actp.tile([P, KO, SH], bf16, tag="zT", name="x2b")

        # FFN1 for chunk 3 (ACT relu, post-exp) with the first w2 block's
        # loads interleaved so FFN2 can start right after
        w2b0 = w2p.tile([P, 8, D], bf16, tag="w2blk", bufs=2, name="w2b0")
        for j in range(JB):
            w1b = w1_load(j)
            nc.sync.dma_start(w2b0[:, j, :], w2src[:, j, :])
            for mo in range(4):
                ffn1_unit(3, j, mo, w1b, hTs[3], relu_on_act=True)

        lnp = []

        def pump_ln(n):
            i = 0
            while lnp and i < n:
                g = lnp[0]
                try:
                    next(g)
                    lnp.append(lnp.pop(0))
                except StopIteration:
                    lnp.pop(0)
                i += 1

        for jj in range(4):
            if jj == 0:
                w2b = w2b0
            else:
                w2b = w2p.tile([P, 8, D], bf16, tag="w2blk", bufs=2, name=f"w2b{jj}")
                nc.sync.dma_start(
                    w2b[:, 0:4, :], w2src[:, 8 * jj : 8 * jj + 4, :]
                )
                nc.sync.dma_start(
                    w2b[:, 4:8, :], w2src[:, 8 * jj + 4 : 8 * jj + 8, :]
                )
            for no in range(NO4):
                tq = slice(no * TQ, (no + 1) * TQ)
                for mo in range(KO):
                    ps = psC.tile([P, TQ], f32, tag="pc", name="ps_f2")
                    for ko8 in range(8):
                        nc.tensor.matmul(
                            ps[:],
                            w2b[:, ko8, mo * P : (mo + 1) * P],
                            hTs[no][:, 8 * jj + ko8, :],
                            start=(ko8 == 0),
                            stop=(ko8 == 7),
                        )
                    if jj == 0:
                        t2 = ffn.tile([P, TQ], f32, tag="t2", name="t2f")
                        nc.vector.tensor_scalar(
                            t2[:], zT[:, mo, tq], g1_sb[:, mo : mo + 1],
                            be1b2_sb[:, mo : mo + 1], ALU.mult, ALU.add,
                        )
                        nc.vector.tensor_add(x2a[:, mo, tq], ps[:], t2[:])
                    elif jj < 3:
                        nc.vector.tensor_add(
                            x2a[:, mo, tq], x2a[:, mo, tq], ps[:]
                        )
                    else:
                        nc.vector.tensor_add(
                            x2b[:, mo, tq], x2a[:, mo, tq], ps[:]
                        )
                    if jj == 3:
                        pump_ln(2)
                if jj == 3:
                    if no % 2 == 1:
                        lnp.append(ln_chunk(x2b, no - 1, emit_out, w2p, "A"))
                        lnp.append(ln_chunk(x2b, no, emit_out, w2p, "B"))
                    pump_ln(4)
        pump_ln(10**9)

        w2p_cm.__exit__(None, None, None)
        ffn_cm.__exit__(None, None, None)
        actp_cm.__exit__(None, None, None)
        psC_cm.__exit__(None, None, None)
        psB_cm.__exit__(None, None, None)
        psA_cm.__exit__(None, None, None)
        misc_cm.__exit__(None, None, None)

    return nc


_NC_CACHE = None


def _get_nc():
    global _NC_CACHE
    if _NC_CACHE is None:
        _NC_CACHE = build_nc()
    return _NC_CACHE


def make_in_maps(inputs):
    f = lambda a: np.ascontiguousarray(np.asarray(a, np.float32))
    fp8 = ml_dtypes.float8_e4m3
    b16 = ml_dtypes.bfloat16
    x = f(inputs["x"])
    Wk, Wv, Wq, Wo = f(inputs["Wk"]), f(inputs["Wv"]), f(inputs["Wq"]), f(inputs["Wo"])
    W1, W2 = f(inputs["W1"]), f(inputs["W2"])
    bv, bo = f(inputs["bv"]), f(inputs["bo"])
    b1, b2 = f(inputs["b1"]), f(inputs["b2"])
    g1, be1 = f(inputs["g1"]), f(inputs["be1"])
    shared = {
        "wkT8": np.ascontiguousarray((Wk.T * 64.0).astype(fp8)),
        "wvT8": np.ascontiguousarray((Wv.T * 64.0).astype(fp8)),
        "wqT8": np.ascontiguousarray((Wq.T * 64.0).astype(fp8)),
        "woT8": np.ascontiguousarray((Wo.T * 64.0).astype(fp8)),
        "w1T": np.ascontiguousarray((W1 * g1[None, :]).T.astype(b16)),
        "w2T": np.ascontiguousarray(W2.T.astype(b16)),
        "bk": f(inputs["bk"]),
        "bq": f(inputs["bq"]),
        "bo2": bo + Wo @ bv,
        "b1f": b1 + W1 @ be1,
        "be1b2": be1 + b2,
        "g1": g1,
        "g2": f(inputs["g2"]),
        "be2": f(inputs["be2"]),
    }
    in_maps = []
    for c in range(8):
        b, g = c // 2, c % 2
        xT = np.ascontiguousarray(x[b].T)
        xT8 = xT.astype(fp8)
        in_maps.append(
            {
                "xT8": xT8,
                "xh8": np.ascontiguousarray(xT8[:, g * SH : (g + 1) * SH]),
                "xhT": np.ascontiguousarray(xT[:, g * SH : (g + 1) * SH].astype(b16)),
                **shared,
            }
        )
    return in_maps


def assemble(results):
    out = np.empty((4, S, D), np.float32)
    for c in range(8):
        b, g = c // 2, c % 2
        out[b, g * SH : (g + 1) * SH, :] = results[c]["out"].T
    return out


def kernel(**inputs):
    nc = _get_nc()
    res = run_bass_kernel_spmd(nc, make_in_maps(inputs), list(range(8)))
    return assemble(res.results)
